# revision 9
# baseline (speedup 1.0000x reference)
"""Trainium2 Bass kernel for CausalMessagePassingLayer (2x GCN + gated scatter).

Sharding: 8 cores = 4 samples x 2 halves of the embedding dim (D=768 -> 384).
Each core is fully independent (no collectives).

Sparse aggregation design (replaces the dense 4096x4096 adjacency matmuls):
  - host pre-gathers x^T = t[t2x]^T (pure input marshalling), so the xw
    matmul needs no on-device gather: y = dinv_src * (x @ W[:, half])
  - y rows (bf16) go to DRAM; edges are dst-sorted on the host and padded
    per 128-wide dst tile to a uniform chunk count C; dma_gather pulls the
    y[src] row per edge slot
  - per 128-edge chunk, a one-hot scatter matrix M[e, dst_local] is built
    on-device (iota == dst_local compare) and z_tile += M^T @ y_edges
    accumulates in f32 PSUM; self-loops are one identity matmul per tile
    straight out of SBUF (no gather, and keeps the pad count C at 9)
  - e = dinv_dst*tanh(gate)*z + tanh(gate)*bias -> bf16 rows
  - out starts as t_half (base copy) and both GCNs' e rows are
    dma_scatter_add-ed onto it at their token positions

Host-side work is restricted to index/descriptor preparation (sorting edges,
degree counts, index wrapping) and dtype/layout marshalling of inputs; all
floating-point math runs on device.
"""

import numpy as np
import ml_dtypes

B, S, D, N, E = 4, 8192, 768, 4096, 32768
H = D // 2            # per-core half of embedding dim
P = 128
NT = N // P           # 32 dst tiles per graph
KC = D // P           # 6 contraction chunks for xw
XW = 512              # tokens per xw wave
GT = 2                # dst tiles per gather call
EW = 8                # dst tiles per e-scatter wave
PADLOC = 200.0        # out-of-range dst_local for padded edge slots

bf16 = ml_dtypes.bfloat16

# test-harness knobs (the grading harness just calls kernel() and these stay default)
DBG_NO_SCATTER = False
TRACE = False
TRACE_CORES = None
LAST_RESULT = None


def _wrap_idx(idx):
    """dma_gather/scatter index layout: i -> [i % 16, i // 16], replicated to 128."""
    n = idx.shape[0]
    assert n % 16 == 0
    w = idx.astype(np.int16).reshape(n // 16, 16).T
    return np.ascontiguousarray(np.tile(w, (8, 1)))


def _prep_edges(ei, C):
    """dst-sorted edge slots padded per dst tile to C chunks of 128.

    Returns (src_flat[NT*C*P] int64, dloc_col[P, NT*C] f32, deg[N] int32).
    Padded slots get src=0 (gathered data is ignored) and dst_local=PADLOC
    (one-hot row is all zeros). deg includes the self loop.
    """
    src, dst = ei[0].astype(np.int64), ei[1].astype(np.int64)
    deg = (np.bincount(dst, minlength=N) + 1).astype(np.int32)
    order = np.argsort(dst, kind="stable")
    srcs, dsts = src[order], dst[order]
    cnt = np.bincount(dsts // P, minlength=NT)
    assert cnt.max() <= C * P
    offs = np.concatenate([[0], np.cumsum(cnt)])
    src_pad = np.zeros((NT, C * P), np.int64)
    dloc_pad = np.full((NT, C * P), PADLOC, np.float32)
    for t in range(NT):
        k = cnt[t]
        src_pad[t, :k] = srcs[offs[t] : offs[t + 1]]
        dloc_pad[t, :k] = dsts[offs[t] : offs[t + 1]] - t * P
    dloc_col = np.ascontiguousarray(dloc_pad.reshape(NT * C, P).T)
    return src_pad.reshape(-1), dloc_col, deg


def _core_data(inputs):
    t_full = np.asarray(inputs["token_embeddings"], dtype=np.float32)
    W = {
        "e": np.asarray(inputs["W_edges"], dtype=np.float32),
        "n": np.asarray(inputs["W_nodes"], dtype=np.float32),
    }
    bias = {
        "e": np.asarray(inputs["b_edges"], dtype=np.float32),
        "n": np.asarray(inputs["b_nodes"], dtype=np.float32),
    }
    gate = {
        "e": np.asarray(inputs["gate_a"], dtype=np.float32).reshape(1, 1),
        "n": np.asarray(inputs["gate_b"], dtype=np.float32).reshape(1, 1),
    }
    t2x = {
        "e": np.asarray(inputs["tokens2edges"], dtype=np.int64),
        "n": np.asarray(inputs["tokens2nodes"], dtype=np.int64),
    }
    x2t = {
        "e": np.asarray(inputs["edges2tokens"], dtype=np.int64),
        "n": np.asarray(inputs["nodes2tokens"], dtype=np.int64),
    }
    ei = {
        "e": np.asarray(inputs["edge_index_edges"], dtype=np.int64),
        "n": np.asarray(inputs["edge_index_nodes"], dtype=np.int64),
    }

    gcns = ("e", "n")

    # uniform chunk count across all cores (SPMD: one program for all)
    C = 0
    for g in gcns:
        for b in range(B):
            cnt = np.bincount(ei[g][b][1] // P, minlength=NT)
            C = max(C, int(np.ceil(cnt.max() / P)))
    NCH = NT * C            # chunks per GCN
    NS = NCH * P            # edge slots per GCN

    edge_prep = {g: [_prep_edges(ei[g][b], C) for b in range(B)] for g in gcns}

    # ---- per-core host data ----
    iota_row = np.arange(P, dtype=np.float32).reshape(1, P)
    pidx_col = np.arange(P, dtype=np.float32).reshape(P, 1)
    core_data = []
    for b in range(B):
        for h in range(2):
            d = dict(iota=iota_row, pidx=pidx_col)
            d["t_half"] = np.ascontiguousarray(
                t_full[b][:, h * H : (h + 1) * H]
            ).astype(bf16)
            for g in gcns:
                src_flat, dloc_col, deg = edge_prep[g][b]
                x = t_full[b][t2x[g][b]].astype(bf16)
                d[f"xT_{g}"] = np.ascontiguousarray(
                    x.T.reshape(KC, P, N).transpose(1, 0, 2)
                )
                d[f"W_{g}"] = np.ascontiguousarray(
                    W[g][:, h * H : (h + 1) * H].reshape(KC, P, H).transpose(1, 0, 2)
                ).astype(bf16)
                d[f"bias_{g}"] = np.ascontiguousarray(bias[g][None, h * H : (h + 1) * H])
                d[f"gate_{g}"] = gate[g]
                d[f"deg_pc_{g}"] = np.ascontiguousarray(deg.reshape(NT, P).T)
                d[f"srcw_{g}"] = _wrap_idx(src_flat)
                d[f"dloc_{g}"] = dloc_col
                d[f"x2tw_{g}"] = _wrap_idx(x2t[g][b])
            core_data.append(d)
    return core_data, C


def _build(C, debug=False):
    import concourse.bacc as bacc
    import concourse.mybir as mybir
    import concourse.tile as tile

    f32, bft, i16, i32 = (
        mybir.dt.float32,
        mybir.dt.bfloat16,
        mybir.dt.int16,
        mybir.dt.int32,
    )
    gcns = ("e", "n")
    NCH = NT * C
    NS = NCH * P

    nc = bacc.Bacc("TRN2", target_bir_lowering=False, debug=debug, num_swdge_queues=1)

    ins_d = {}
    ins_d["iota"] = nc.declare_dram_parameter("iota", [1, P], f32, isOutput=False)
    ins_d["pidx"] = nc.declare_dram_parameter("pidx", [P, 1], f32, isOutput=False)
    ins_d["t_half"] = nc.declare_dram_parameter("t_half", [S, H], bft, isOutput=False)
    for g in gcns:
        ins_d[f"xT_{g}"] = nc.declare_dram_parameter(f"xT_{g}", [P, KC, N], bft, isOutput=False)
        ins_d[f"W_{g}"] = nc.declare_dram_parameter(f"W_{g}", [P, KC, H], bft, isOutput=False)
        ins_d[f"bias_{g}"] = nc.declare_dram_parameter(f"bias_{g}", [1, H], f32, isOutput=False)
        ins_d[f"gate_{g}"] = nc.declare_dram_parameter(f"gate_{g}", [1, 1], f32, isOutput=False)
        ins_d[f"deg_pc_{g}"] = nc.declare_dram_parameter(f"deg_pc_{g}", [P, NT], i32, isOutput=False)
        ins_d[f"srcw_{g}"] = nc.declare_dram_parameter(f"srcw_{g}", [P, NS // 16], i16, isOutput=False)
        ins_d[f"dloc_{g}"] = nc.declare_dram_parameter(f"dloc_{g}", [P, NCH], f32, isOutput=False)
        ins_d[f"x2tw_{g}"] = nc.declare_dram_parameter(f"x2tw_{g}", [P, N // 16], i16, isOutput=False)
    out_d = nc.declare_dram_parameter("out", [S, H], bft, isOutput=True)

    y_d = {g: nc.dram_tensor(f"y_{g}", [N, H], bft) for g in gcns}
    edump_d = {g: nc.dram_tensor(f"edump_{g}", [N, H], bft) for g in gcns}

    with tile.TileContext(nc) as tc:
        with (
            tc.tile_pool(name="cst", bufs=1) as cst,
            tc.tile_pool(name="idxp", bufs=1) as idxp,
            tc.tile_pool(name="xt", bufs=2) as xtp,
            tc.tile_pool(name="yp", bufs=1) as yp,
            tc.tile_pool(name="gep", bufs=2) as gep,
            tc.tile_pool(name="mp", bufs=2) as mp,
            tc.tile_pool(name="ep", bufs=2) as ep,
            tc.tile_pool(name="psxw", bufs=2, space="PSUM") as psxw,
            tc.tile_pool(name="psz", bufs=2, space="PSUM") as psz,
        ):
            # ---------- setup ----------
            iota_bc = cst.tile([P, P], f32, name="iota_bc", tag="iota_bc")
            nc.sync.dma_start(out=iota_bc[:], in_=ins_d["iota"][:1, :].to_broadcast([P, P]))
            pidx = cst.tile([P, 1], f32, name="pidx", tag="pidx")
            nc.sync.dma_start(out=pidx[:], in_=ins_d["pidx"][:])
            ident = cst.tile([P, P], bft, name="ident", tag="ident")
            nc.vector.tensor_scalar(
                out=ident[:], in0=iota_bc[:], scalar1=pidx[:, :1], scalar2=None,
                op0=mybir.AluOpType.is_equal,
            )

            # base copy: out starts as t_half (scatter_adds land on top).
            # Bounced through SBUF — direct DRAM->DRAM descriptors crash the DGE.
            for bw in range(2):
                tb = xtp.tile([P, S // P // 2, H], bft, name="tb", tag="tb")
                nc.sync.dma_start(
                    out=tb[:],
                    in_=ins_d["t_half"].rearrange("(c p) h -> p c h", p=P)[
                        :, bw * (S // P // 2) : (bw + 1) * (S // P // 2), :
                    ],
                )
                nc.sync.dma_start(
                    out=out_d.rearrange("(c p) h -> p c h", p=P)[
                        :, bw * (S // P // 2) : (bw + 1) * (S // P // 2), :
                    ],
                    in_=tb[:],
                )

            Wsb, bias_ga, dinv, dinv_ga = {}, {}, {}, {}
            idx_src, idx_x2t, dloc = {}, {}, {}
            for g in gcns:
                Wsb[g] = cst.tile([P, KC, H], bft, name=f"W_{g}", tag=f"W_{g}")
                nc.sync.dma_start(out=Wsb[g][:], in_=ins_d[f"W_{g}"][:])

                gcol = cst.tile([P, 1], f32, name=f"gcol_{g}", tag=f"gcol_{g}")
                nc.sync.dma_start(
                    out=gcol[:], in_=ins_d[f"gate_{g}"][:1, :].to_broadcast([P, 1])
                )
                tanh_g = cst.tile([P, 1], f32, name=f"tanh_{g}", tag=f"tanh_{g}")
                nc.scalar.activation(
                    out=tanh_g[:], in_=gcol[:], func=mybir.ActivationFunctionType.Tanh
                )

                brow = cst.tile([P, H], f32, name=f"brow_{g}", tag=f"brow_{g}")
                nc.sync.dma_start(
                    out=brow[:], in_=ins_d[f"bias_{g}"][:1, :].to_broadcast([P, H])
                )
                bias_ga[g] = cst.tile([P, H], f32, name=f"biasga_{g}", tag=f"biasga_{g}")
                nc.vector.tensor_scalar_mul(bias_ga[g][:], brow[:], tanh_g[:, :1])

                deg_i = cst.tile([P, NT], i32, name=f"degi_{g}", tag=f"degi_{g}")
                nc.sync.dma_start(out=deg_i[:], in_=ins_d[f"deg_pc_{g}"][:])
                deg_f = cst.tile([P, NT], f32, name=f"degf_{g}", tag=f"degf_{g}")
                nc.vector.tensor_copy(out=deg_f[:], in_=deg_i[:])
                rdeg = cst.tile([P, NT], f32, name=f"rdeg_{g}", tag=f"rdeg_{g}")
                nc.vector.reciprocal(rdeg[:], deg_f[:])
                dinv[g] = cst.tile([P, NT], f32, name=f"dinv_{g}", tag=f"dinv_{g}")
                nc.scalar.sqrt(dinv[g][:], rdeg[:])
                dinv_ga[g] = cst.tile([P, NT], f32, name=f"dinvga_{g}", tag=f"dinvga_{g}")
                nc.vector.tensor_scalar_mul(dinv_ga[g][:], dinv[g][:], tanh_g[:, :1])

                idx_src[g] = idxp.tile([P, NS // 16], i16, name=f"isrc_{g}", tag=f"isrc_{g}")
                nc.sync.dma_start(out=idx_src[g][:], in_=ins_d[f"srcw_{g}"][:])
                idx_x2t[g] = idxp.tile([P, N // 16], i16, name=f"ix2t_{g}", tag=f"ix2t_{g}")
                nc.sync.dma_start(out=idx_x2t[g][:], in_=ins_d[f"x2tw_{g}"][:])
                dloc[g] = idxp.tile([P, NCH], f32, name=f"dloc_{g}", tag=f"dloc_{g}")
                nc.sync.dma_start(out=dloc[g][:], in_=ins_d[f"dloc_{g}"][:])

            # ---------- xw: y = dinv_src * (x @ W) ----------
            y_sb = {}
            for g in gcns:
                y_sb[g] = yp.tile([P, NT, H], bft, name=f"y_{g}", tag=f"y_{g}")
                for w in range(N // XW):
                    xt = xtp.tile([P, KC, XW], bft)
                    nc.sync.dma_start(
                        out=xt[:], in_=ins_d[f"xT_{g}"][:, :, w * XW : (w + 1) * XW]
                    )
                    for c4 in range(XW // P):
                        c = w * (XW // P) + c4
                        ps = psxw.tile([P, H], f32)
                        for k in range(KC):
                            nc.tensor.matmul(
                                out=ps[:],
                                lhsT=xt[:, k, c4 * P : (c4 + 1) * P],
                                rhs=Wsb[g][:, k, :],
                                start=(k == 0),
                                stop=(k == KC - 1),
                            )
                        nc.scalar.activation(
                            out=y_sb[g][:, c, :],
                            in_=ps[:],
                            func=mybir.ActivationFunctionType.Copy,
                            scale=dinv[g][:, c : c + 1],
                        )
                nc.sync.dma_start(
                    out=y_d[g].rearrange("(c p) h -> p c h", p=P), in_=y_sb[g][:]
                )

            # ---------- sparse aggregation + gated scatter ----------
            CPG = 8          # chunks per gather call: 1024 idxs = SWDGE carveout cap
            assert NCH % CPG == 0
            for g in gcns:
                ew = None
                ge_tiles = {}
                for t in range(NT):
                    if t % EW == 0:
                        ew = ep.tile([P, EW, H], bft, name="ew", tag="ew")
                    zt = psz.tile([P, H], f32, name="zt", tag="zt")
                    nc.tensor.matmul(
                        out=zt[:], lhsT=ident[:], rhs=y_sb[g][:, t, :],
                        start=True, stop=False,
                    )
                    Mt = mp.tile([P, C, P], bft, name="Mt", tag="Mt")
                    for j in range(C):
                        c = t * C + j
                        ci, sl = c // CPG, c % CPG
                        if ci not in ge_tiles:
                            ge = gep.tile([P, CPG, H], bft, name="ge", tag="ge")
                            nc.gpsimd.dma_gather(
                                out_ap=ge[:],
                                in_ap=y_d[g][:],
                                idxs_ap=idx_src[g][:, ci * (CPG * P // 16) : (ci + 1) * (CPG * P // 16)],
                                num_idxs=CPG * P,
                                num_idxs_reg=CPG * P,
                                elem_size=H,
                                queue_num=0,
                            )
                            ge_tiles[ci] = ge
                        nc.vector.tensor_scalar(
                            out=Mt[:, j, :], in0=iota_bc[:],
                            scalar1=dloc[g][:, c : c + 1],
                            scalar2=None, op0=mybir.AluOpType.is_equal,
                        )
                        nc.tensor.matmul(
                            out=zt[:], lhsT=Mt[:, j, :],
                            rhs=ge_tiles[ci][:, sl, :],
                            start=False, stop=(j == C - 1),
                        )
                    nc.vector.scalar_tensor_tensor(
                        out=ew[:, t % EW, :],
                        in0=zt[:],
                        scalar=dinv_ga[g][:, t : t + 1],
                        in1=bias_ga[g][:],
                        op0=mybir.AluOpType.mult,
                        op1=mybir.AluOpType.add,
                    )
                    if t % EW == EW - 1:
                        wv = t // EW
                        if DBG_NO_SCATTER:
                            nc.sync.dma_start(
                                out=edump_d[g].rearrange("(c p) h -> p c h", p=P)[
                                    :, wv * EW : (wv + 1) * EW, :
                                ],
                                in_=ew[:],
                            )
                        else:
                            nc.gpsimd.dma_scatter_add(
                                out_ap=out_d[:],
                                in_ap=ew[:],
                                idxs_ap=idx_x2t[g][:, wv * (EW * P // 16) : (wv + 1) * (EW * P // 16)],
                                num_idxs=EW * P,
                                num_idxs_reg=EW * P,
                                elem_size=H,
                                queue_num=0,
                            )

    nc.compile()
    return nc


def kernel(**inputs):
    from concourse.bass_utils import run_bass_kernel_spmd

    core_data, C = _core_data(inputs)
    nc = _build(C)

    in_maps = [{k: v for k, v in cd.items()} for cd in core_data]
    global LAST_RESULT
    kw = {}
    if TRACE:
        kw = dict(trace=True, trace_cores=TRACE_CORES, stitch_traces=False)
    res = run_bass_kernel_spmd(nc, in_maps, list(range(8)), **kw)
    LAST_RESULT = res

    out = np.empty((B, S, D), np.float32)
    for b in range(B):
        for h in range(2):
            o = np.asarray(res.results[2 * b + h]["out"])
            out[b, :, h * H : (h + 1) * H] = o.astype(np.float32)
    return out
